# revision 26
# baseline (speedup 1.0000x reference)
"""Trainium2 Bass kernel for the GAT-gate GNN forward pass.

Data-parallel over the batch axis: 16 graphs across 8 NeuronCores (2 each).

Per-graph math (N=1024, D=128, NHOP=4), matching reference.py:
    h   = x @ W_w.T + W_b
    hA  = h @ A
    e   = hA @ h.T;  e_sym = e + e.T            (symmetric)
    l   = where(adj>0, e_sym, 0)
    att = softmax(l, axis=0-of-rows) * adj       (softmax over the row index)
    z = h; repeat 4x:
        az = relu(att @ z)
        c  = sigmoid(x @ gw_x + az @ gw_az + gb)
        z  = c*x + (1-c)*az

Device-side structure (v5):
  * ONE PSUM pool: 4 slots of [128,1024] (2 banks each = all 8 banks).
    Phase 1 (h/hA), phase 2 (e_sym), phase 3 (h^T transposes) and phase 4
    (az) all ride the same slot rotation, giving a 4-deep phase-2
    pipeline with no separate-pool fragmentation.
  * e_sym per row-tile r: the additive mask (0 / -1024) is sent as
    fp8e5m2 (exact) and injected into PSUM by two fp8 DoubleRow matmuls
    (106ns each) against constant [I;0] / [0;I] selector weights; two
    fp16 matmuls per 512-chunk accumulate hA_r.h + h_r.hA on top.
    exp flushes masked logits to exactly 0 in fp16; the reference
    denominator's cnt0*exp(0-m) terms are restored from a host-sent
    non-edge count (corr).
  * one merged DVE rowmax reduce [128,1024] (negate=True -> exp bias) and
    one merged ACT exp -> softmax numerators P16 (fp16, <= 1).  Row sums
    come from the exp's accum_out on even rows and a Pool-engine
    sum (scalar_tensor_tensor accum) over P16 on odd rows, balancing ACT
    vs the otherwise idle Pool.  Softmax denominators are folded into the
    per-hop z scaling, so there is no normalize pass over P16.
  * z0 = recip * h^T via 8 PE transposes into a pool slot, evicted with
    per-node scaled copies split between ACT and DVE.
  * per hop/graph/half unit: 32 matmuls -> az PSUM; ACT relu evict (a_h);
    gate dot-products on the Pool engine (scalar_tensor_tensor accum);
    sigmoid on ACT (bias=gb); za = a_h*(recip-cs) split ACT/DVE;
    z' = cs*x + za on DVE.  Units are emitted in ready-time order (each
    unit's za/zw block deferred past the next unit's relu) and the two
    graphs are phase-interleaved so every engine's in-order queue matches
    dependency order and PE never waits.
  * all inputs stream over HWDGE DMAs (SP + one early ACT-issued xt0);
    zout is written fp16 per half and upcast on the host, with stores
    alternating between the SP and Pool DMA queues to parallelize the
    drain.
"""

import sys
import numpy as np

sys.path.insert(0, "/opt/trn_rl_repo")

B, N, D, NHOP = 16, 1024, 128, 4
N_CORES = 8
GPC = B // N_CORES  # graphs per core
NT = N // 128       # 8 node tiles
MASK_NOEDGE = -1024.0

_prog_cache = {}
_last_in_maps = None


def _split_sync_waits(nc, max_waits=1):
    """This walrus build rejects instructions carrying more than a couple of
    sync waits; move excess waits onto same-engine NOPs inserted before the
    offending instruction (per-engine program order is preserved)."""
    import concourse.mybir as mybir

    for fn in nc.m.functions:
        for bb in fn.blocks:
            insts = list(bb.instructions)
            out = []
            changed = False
            for inst in insts:
                si = inst.sync_info
                if si is not None and len(si.on_wait) > max_waits:
                    waits = list(si.on_wait)
                    for w in waits[:-max_waits]:
                        n = mybir.InstNoOp(
                            name=nc.get_next_instruction_name(), ins=[], outs=[]
                        )
                        n.engine = inst.engine
                        n.sync_info = mybir.SyncInfo(on_wait=[w], on_update=[])
                        nc.register_instruction(n)
                        out.append(n)
                    inst.sync_info = mybir.SyncInfo(
                        on_wait=waits[-max_waits:], on_update=list(si.on_update)
                    )
                    changed = True
                out.append(inst)
            if changed:
                bb.instructions = out


def _build_program():
    import concourse.bass as bass
    import concourse.tile as tile
    from concourse import mybir
    from concourse.masks import make_identity
    from contextlib import ExitStack

    f32 = mybir.dt.float32
    f16 = mybir.dt.float16
    f8 = mybir.dt.float8e5
    AF = mybir.ActivationFunctionType
    ALU = mybir.AluOpType
    DR = mybir.MatmulPerfMode.DoubleRow

    nc = bass.Bass("TRN2", target_bir_lowering=False, debug=False)

    # ---- DRAM I/O (per core: GPC graphs) ----
    xt_d = nc.dram_tensor("xt16", [GPC, 128, N], f16, kind="ExternalInput")
    x16_d = nc.dram_tensor("x16", [GPC, 128, NT, 128], f16, kind="ExternalInput")
    mask_d = nc.dram_tensor("mask8", [GPC, NT, 128, N], f8, kind="ExternalInput")
    corr_d = nc.dram_tensor("corr", [GPC, 128, NT], f32, kind="ExternalInput")
    wwt_d = nc.dram_tensor("wwt16", [128, 128], f16, kind="ExternalInput")
    amat_d = nc.dram_tensor("amat16", [128, 128], f16, kind="ExternalInput")
    wb_d = nc.dram_tensor("wb", [128, 1], f32, kind="ExternalInput")
    gwx_d = nc.dram_tensor("gwx16", [1, 128], f16, kind="ExternalInput")
    gwaz_d = nc.dram_tensor("gwaz16", [1, 128], f16, kind="ExternalInput")
    gb_d = nc.dram_tensor("gbvec", [128, 1], f32, kind="ExternalInput")
    zout_d = nc.dram_tensor("zout16", [GPC, 2, 128, NT // 2, 128], f16,
                            kind="ExternalOutput")

    with tile.TileContext(nc) as tc:
        with ExitStack() as ctx:
            singles = ctx.enter_context(tc.tile_pool(name="singles", bufs=1))
            gpool = ctx.enter_context(tc.tile_pool(name="gpool", bufs=1))
            masks = ctx.enter_context(tc.tile_pool(name="masks", bufs=2 * NT))
            zpool = ctx.enter_context(tc.tile_pool(name="zpool", bufs=2))
            scratch = ctx.enter_context(tc.tile_pool(name="scratch", bufs=3))
            zasc = ctx.enter_context(tc.tile_pool(name="zasc", bufs=4))
            tiny = ctx.enter_context(tc.tile_pool(name="tiny", bufs=3))
            psbig = ctx.enter_context(tc.tile_pool(name="psbig", bufs=4, space="PSUM"))

            # ---- input loads: xt0 (split in half for first-matmul latency)
            # on the ACT HWDGE queue ahead of the SP param flood ----
            xt0 = gpool.tile([128, N], f16, tag="xt0", name="xt0")
            nc.scalar.dma_start(xt0[:], xt_d.ap()[0])
            wwt = singles.tile([128, 128], f16)
            nc.sync.dma_start(wwt[:], wwt_d.ap())
            wb = singles.tile([128, 1], f32)
            nc.sync.dma_start(wb[:], wb_d.ap())
            xt1 = gpool.tile([128, N], f16, tag="xt1", name="xt1")
            nc.sync.dma_start(xt1[:], xt_d.ap()[1])
            amat = singles.tile([128, 128], f16)
            nc.sync.dma_start(amat[:], amat_d.ap())
            xt_l = [xt0, xt1]
            mask_t = [[None] * NT for _ in range(GPC)]
            for r in range(2):
                mt = masks.tile([128, N], f8, tag="mask", name="mt")
                nc.sync.dma_start(mt[:], mask_d.ap()[0, r])
                mask_t[0][r] = mt
            gb = singles.tile([128, 1], f32)
            nc.sync.dma_start(gb[:], gb_d.ap())
            # gate weight rows broadcast to all 128 partitions via stride-0 DMA
            gwx = singles.tile([128, 128], f16)
            gwx_src = gwx_d.ap()
            nc.sync.dma_start(
                gwx[:],
                bass.AP(tensor=gwx_src.tensor, offset=gwx_src.offset,
                        ap=[[0, 128]] + list(gwx_src.ap[1:])),
            )
            gwaz = singles.tile([128, 128], f16)
            gwaz_src = gwaz_d.ap()
            nc.sync.dma_start(
                gwaz[:],
                bass.AP(tensor=gwaz_src.tensor, offset=gwaz_src.offset,
                        ap=[[0, 128]] + list(gwaz_src.ap[1:])),
            )
            x16_l, corr_l = [], []
            for g in range(GPC):
                x16 = gpool.tile([128, NT, 128], f16, tag=f"x16{g}",
                                 name=f"x16{g}")
                nc.sync.dma_start(x16[:], x16_d.ap()[g])
                corr = gpool.tile([128, NT], f32, tag=f"corr{g}",
                                  name=f"corr{g}")
                nc.sync.dma_start(corr[:], corr_d.ap()[g])
                x16_l.append(x16)
                corr_l.append(corr)
            for g in range(GPC):
                for r in range(2 if g == 0 else 0, NT):
                    mt = masks.tile([128, N], f8, tag="mask", name="mt")
                    nc.sync.dma_start(mt[:], mask_d.ap()[g, r])
                    mask_t[g][r] = mt

            ident = singles.tile([128, 128], f16)
            make_identity(nc, ident)
            # fp8 DoubleRow selector weights: sel0 = [I; 0], sel1 = [0; I]
            sel0 = singles.tile([128, 2, 128], f8)
            nc.gpsimd.memset(sel0[:], 0)
            nc.gpsimd.tensor_copy(sel0[:, 0, :], ident[:])
            sel1 = singles.tile([128, 2, 128], f8)
            nc.gpsimd.memset(sel1[:], 0)
            nc.gpsimd.tensor_copy(sel1[:, 1, :], ident[:])

            # ---- phase 1: g0's h/hA chain first (it gates phase 2); g1's
            # matmuls fill PE bubbles and its ACT evicts drop into the
            # phase-2 exp pipeline's early gaps ----
            h16_l, ha16_l, gx_l = [], [], []
            for g in range(GPC):
                h16_l.append(gpool.tile([128, N], f16, tag=f"h16{g}",
                                        name=f"h16{g}"))
                ha16_l.append(gpool.tile([128, N], f16, tag=f"ha16{g}",
                                         name=f"ha16{g}"))

            def emit_h(g, evict=True):
                ph = psbig.tile([128, 1024], f32, tag="es", name="ph")
                for cc in range(2):
                    sl = slice(cc * 512, (cc + 1) * 512)
                    nc.tensor.matmul(ph[:, sl], wwt[:], xt_l[g][:, sl],
                                     start=True, stop=True)
                    if evict:
                        nc.scalar.activation(h16_l[g][:, sl], ph[:, sl],
                                             AF.Identity, bias=wb[:],
                                             scale=1.0)
                return ph

            def emit_h_evict(g, ph):
                for cc in range(2):
                    sl = slice(cc * 512, (cc + 1) * 512)
                    nc.scalar.activation(h16_l[g][:, sl], ph[:, sl],
                                         AF.Identity, bias=wb[:], scale=1.0)

            def emit_ha(g):
                pa = psbig.tile([128, 1024], f32, tag="es", name="pa")
                for cc in range(2):
                    sl = slice(cc * 512, (cc + 1) * 512)
                    nc.tensor.matmul(pa[:, sl], amat[:], h16_l[g][:, sl],
                                     start=True, stop=True)
                    nc.scalar.copy(ha16_l[g][:, sl], pa[:, sl])

            emit_h(0)
            emit_ha(0)
            ph1 = emit_h(1, evict=False)

            def emit_gx(g):
                gx_all = tiny.tile([128, NT], f32, tag=f"gx{g}", bufs=1,
                                   name=f"gx{g}")
                for t in range(NT):
                    sc = zasc.tile([128, 128], f16, tag="gsc", name="sc",
                                   bufs=2)
                    nc.vector.scalar_tensor_tensor(
                        out=sc[:], in0=x16_l[g][:, t, :], scalar=1.0, in1=gwx[:],
                        op0=ALU.mult, op1=ALU.mult, accum_out=gx_all[:, t:t + 1])
                gx_l.append(gx_all)

            # ---- phase 2/3 state ----
            p16_l, recip_l, nmax_l, ssum_l, z_l = [], [], [], [], []
            for g in range(GPC):
                p16 = gpool.tile([128, NT, N], f16, tag=f"p16{g}",
                                 name=f"p16{g}")
                p16_l.append(p16)
                recip_l.append(tiny.tile([128, NT], f32, tag=f"recip{g}",
                                         bufs=1, name=f"recip{g}"))
                nmax_l.append(tiny.tile([128, NT], f32, tag=f"nmax{g}",
                                        bufs=1, name=f"nmax{g}"))
                ssum_l.append(tiny.tile([128, NT], f32, tag=f"ssum{g}",
                                        bufs=1, name=f"ssum{g}"))
                z_l.append(None)

            def phase2_inject(g, r):
                # fp8 DoubleRow: es[:, :512] = I.T@mask[:,0,:] + 0,
                #                es[:, 512:] = 0 + I.T@mask[:,1,:]
                mt = mask_t[g][r]
                m2 = mt[:].rearrange("p (two n) -> p two n", two=2)
                es = psbig.tile([128, 1024], f32, tag="es", name="es")
                nc.tensor.matmul(es[:, 0:512], sel0[:], m2,
                                 start=True, stop=False, perf_mode=DR)
                nc.tensor.matmul(es[:, 512:1024], sel1[:], m2,
                                 start=True, stop=False, perf_mode=DR)
                return es

            def phase2_row(g, r, es=None):
                h16, ha16 = h16_l[g], ha16_l[g]
                if es is None:
                    es = phase2_inject(g, r)
                rc = slice(r * 128, (r + 1) * 128)
                for cc in range(2):
                    sl = slice(cc * 512, (cc + 1) * 512)
                    nc.tensor.matmul(es[:, sl], ha16[:, rc], h16[:, sl],
                                     start=False, stop=False)
                    nc.tensor.matmul(es[:, sl], h16[:, rc], ha16[:, sl],
                                     start=False, stop=True)
                nmax = nmax_l[g][:, r:r + 1]
                nc.vector.tensor_reduce(nmax, es[:], mybir.AxisListType.X,
                                        ALU.max, negate=True)
                nc.scalar.activation(p16_l[g][:, r, :], es[:], AF.Exp,
                                     bias=nmax, scale=1.0,
                                     accum_out=ssum_l[g][:, r:r + 1])

            def phase3_den(g):
                # den = ssum + corr * exp(-rowmax)
                expm = tiny.tile([128, NT], f32, tag="expm", name="expm")
                nc.scalar.activation(expm[:], nmax_l[g][:], AF.Exp)
                t0 = tiny.tile([128, NT], f32, tag="t0", name="t0")
                nc.gpsimd.tensor_mul(t0[:], expm[:], corr_l[g][:])
                den = tiny.tile([128, NT], f32, tag="den", name="den")
                nc.gpsimd.tensor_add(den[:], t0[:], ssum_l[g][:])
                nc.vector.reciprocal(recip_l[g][:], den[:])

            def phase3_z0_transposes(g):
                pt = psbig.tile([128, NT, 128], f16, tag="es", name="pt")
                for t in range(NT):
                    nc.tensor.transpose(pt[:, t, :],
                                        h16_l[g][:, t * 128:(t + 1) * 128],
                                        ident[:])
                z_cur = zpool.tile([128, NT, 128], f16, tag=f"z{g}",
                                   name=f"z0_{g}")
                z_l[g] = z_cur
                return pt

            def phase3_z0_evict(g, pt, ts):
                z_cur = z_l[g]
                for t in ts:
                    if t % 2 == 0:
                        nc.scalar.mul(z_cur[:, t, :], pt[:, t, :],
                                      recip_l[g][:, t:t + 1])
                    else:
                        nc.vector.tensor_scalar_mul(z_cur[:, t, :], pt[:, t, :],
                                                    recip_l[g][:, t:t + 1])

            # ---- phase 2 emission: row r0 starts as soon as g0's h/hA are
            # out; g1's phase-1 evictions slot into the exp pipeline gaps ----
            es00 = phase2_inject(0, 0)
            es01 = phase2_inject(0, 1)
            phase2_row(0, 0, es00)
            emit_h_evict(1, ph1)
            phase2_row(0, 1, es01)
            emit_ha(1)
            for r in range(2, NT):
                phase2_row(0, r)
            # g1 rows with g0's phase 3 interleaved into the pipeline tail
            pt0 = None
            for r in range(NT):
                phase2_row(1, r)
                if r == 1:
                    phase3_den(0)
                elif r == 2:
                    pt0 = phase3_z0_transposes(0)
                elif r == 3:
                    phase3_z0_evict(0, pt0, range(0, 4))
                elif r == 4:
                    phase3_z0_evict(0, pt0, range(4, NT))
            emit_gx(0)
            emit_gx(1)
            phase3_den(1)

            # ---- phase 4 ----
            # Per unit (g, half): PE az matmuls; ACT relu evict + sigmoid;
            # DVE gate dot-products (stt accum) + blend writes; Pool the
            # za = a_h*(recip-cs) scalings (walrus allows tensor_scalar with
            # a per-partition pointer on GPSIMD, but not scalar_tensor_tensor
            # or PSUM access).  Each engine's in-order queue then matches
            # dependency order naturally.
            def emit_unit(g, half, z_next, last, interleave=None):
                p16, gx_all, x16 = p16_l[g], gx_l[g], x16_l[g]
                z_cur = z_l[g]
                az_h = psbig.tile([128, 512], f32, tag="es", name="az")
                for i2 in range(4):
                    i = half * 4 + i2
                    sl = slice(i2 * 128, (i2 + 1) * 128)
                    for j in range(NT):
                        nc.tensor.matmul(az_h[:, sl],
                                         p16[:, j, i * 128:(i + 1) * 128],
                                         z_cur[:, j, :],
                                         start=(j == 0),
                                         stop=(j == NT - 1))
                if interleave is not None:
                    interleave()
                final = last and g == 1 and half == 1
                a_h = scratch.tile([128, 512], f16, tag="a_h", name="a_h")
                nc.scalar.activation(a_h[:], az_h[:], AF.Relu)
                gaz_h = tiny.tile([128, 4], f32, tag="gazh", name="gazh")
                for i2 in range(4):
                    sc = zasc.tile([128, 128], f16, tag="gsc", name="sc",
                                   bufs=2)
                    if final:
                        # drain-critical: read az PSUM directly (relu via
                        # max-op) so the chain starts before the relu evict
                        nc.vector.scalar_tensor_tensor(
                            out=sc[:], in0=az_h[:, i2 * 128:(i2 + 1) * 128],
                            scalar=0.0, in1=gwaz[:],
                            op0=ALU.max, op1=ALU.mult,
                            accum_out=gaz_h[:, i2:i2 + 1])
                    else:
                        nc.vector.scalar_tensor_tensor(
                            out=sc[:], in0=a_h[:, i2 * 128:(i2 + 1) * 128],
                            scalar=1.0, in1=gwaz[:],
                            op0=ALU.mult, op1=ALU.mult,
                            accum_out=gaz_h[:, i2:i2 + 1])
                hs = slice(half * 4, half * 4 + 4)
                sig_in = tiny.tile([128, 4], f32, tag="sigin", name="sigin")
                nc.vector.tensor_add(sig_in[:], gaz_h[:], gx_all[:, hs])
                c_h = tiny.tile([128, 4], f32, tag="c_h", name="c_h")
                nc.scalar.activation(c_h[:], sig_in[:], AF.Sigmoid, bias=gb[:])
                if not last:
                    cs = tiny.tile([128, 4], f32, tag="cs", name="cs")
                    nc.vector.tensor_mul(cs[:], c_h[:], recip_l[g][:, hs])
                    cm1s = tiny.tile([128, 4], f32, tag="cm1s", name="cm1s")
                    nc.vector.tensor_sub(cm1s[:], recip_l[g][:, hs], cs[:])
                else:
                    cs = c_h
                    cm1s = tiny.tile([128, 4], f32, tag="cm1s", name="cm1s")
                    nc.vector.tensor_scalar(
                        out=cm1s[:], in0=c_h[:], scalar1=-1.0,
                        scalar2=1.0, op0=ALU.mult, op1=ALU.add)
                for i2 in range(4):
                    i = half * 4 + i2
                    asl = a_h[:, i2 * 128:(i2 + 1) * 128]
                    # blend engine assignment per chunk (balance ACT/DVE/Pool;
                    # plain tensor_scalar gets DVE 4x mode, stt does not):
                    #   i2=0: za/cx/add all Pool   i2=1: za ACT + stt DVE
                    #   i2=2: za DVE + stt DVE     i2=3: za DVE + cx/add Pool
                    if final:
                        za = zasc.tile([128, 128], f16, tag="zad", name="za")
                        nc.vector.tensor_scalar_mul(za[:], asl,
                                                    cm1s[:, i2:i2 + 1])
                        nc.vector.scalar_tensor_tensor(
                            out=z_next[:, i, :], in0=x16[:, i, :],
                            scalar=cs[:, i2:i2 + 1],
                            in1=za[:], op0=ALU.mult, op1=ALU.add)
                        continue
                    if i2 == 0:
                        za = zasc.tile([128, 128], f16, tag="zap", name="za")
                        nc.gpsimd.tensor_scalar_mul(za[:], asl,
                                                    cm1s[:, i2:i2 + 1])
                        cx = zasc.tile([128, 128], f16, tag="cx", name="cx",
                                       bufs=2)
                        nc.gpsimd.tensor_scalar_mul(cx[:], x16[:, i, :],
                                                    cs[:, i2:i2 + 1])
                        nc.gpsimd.tensor_add(z_next[:, i, :], cx[:], za[:])
                    elif i2 == 1:
                        za = zasc.tile([128, 128], f16, tag="zaa", name="za")
                        nc.scalar.mul(za[:], asl, cm1s[:, i2:i2 + 1])
                        nc.vector.scalar_tensor_tensor(
                            out=z_next[:, i, :], in0=x16[:, i, :],
                            scalar=cs[:, i2:i2 + 1],
                            in1=za[:], op0=ALU.mult, op1=ALU.add)
                    elif i2 == 2:
                        za = zasc.tile([128, 128], f16, tag="zad", name="za")
                        nc.vector.tensor_scalar_mul(za[:], asl,
                                                    cm1s[:, i2:i2 + 1])
                        nc.vector.scalar_tensor_tensor(
                            out=z_next[:, i, :], in0=x16[:, i, :],
                            scalar=cs[:, i2:i2 + 1],
                            in1=za[:], op0=ALU.mult, op1=ALU.add)
                    else:
                        za = zasc.tile([128, 128], f16, tag="zad", name="za")
                        nc.vector.tensor_scalar_mul(za[:], asl,
                                                    cm1s[:, i2:i2 + 1])
                        cx = zasc.tile([128, 128], f16, tag="cx", name="cx",
                                       bufs=2)
                        nc.gpsimd.tensor_scalar_mul(cx[:], x16[:, i, :],
                                                    cs[:, i2:i2 + 1])
                        nc.gpsimd.tensor_add(z_next[:, i, :], cx[:], za[:])
                if last:
                    nc.sync.dma_start(zout_d.ap()[g, half],
                                      z_next[:, half * 4:(half + 1) * 4, :])

            pt1 = None
            for t_hop in range(NHOP):
                last = t_hop == NHOP - 1
                z_next_l = []
                for g in range(GPC):
                    if last:
                        z_next_l.append(gpool.tile([128, NT, 128], f16,
                                                   tag=f"zo{g}",
                                                   name=f"zo{g}"))
                    else:
                        z_next_l.append(zpool.tile([128, NT, 128], f16,
                                                   tag=f"z{g}",
                                                   name=f"zn{g}"))
                for g in range(GPC):
                    for half in range(2):
                        # hop 0 rides the phase-2/3 tail: g1's z0 work is
                        # interleaved between g0's first az units
                        interleave = None
                        if t_hop == 0 and g == 0 and half == 0:
                            def interleave():
                                nonlocal pt1
                                pt1 = phase3_z0_transposes(1)
                        elif t_hop == 0 and g == 0 and half == 1:
                            def interleave():
                                phase3_z0_evict(1, pt1, range(NT))
                        emit_unit(g, half, z_next_l[g], last, interleave)
                    if not last:
                        z_l[g] = z_next_l[g]

    _split_sync_waits(nc)
    return nc


def kernel(x, adj, W_w, W_b, A, gate_w, gate_b):
    import ml_dtypes
    from concourse.bass_utils import run_bass_kernel_spmd

    x = np.asarray(x, dtype=np.float32)
    adj = np.asarray(adj, dtype=np.float32)
    W_w = np.asarray(W_w, dtype=np.float32)
    W_b = np.asarray(W_b, dtype=np.float32)
    A = np.asarray(A, dtype=np.float32)
    gate_w = np.asarray(gate_w, dtype=np.float32)
    gate_b = np.asarray(gate_b, dtype=np.float32)

    if "nc" not in _prog_cache:
        _prog_cache["nc"] = _build_program()
    nc = _prog_cache["nc"]

    # ---- host-side prep ----
    xt16 = np.ascontiguousarray(x.transpose(0, 2, 1)).astype(np.float16)
    x16 = np.ascontiguousarray(
        x.reshape(B, NT, 128, D).transpose(0, 2, 1, 3)).astype(np.float16)
    mask8 = np.where(adj > 0.0, 0.0, MASK_NOEDGE).astype(
        ml_dtypes.float8_e5m2)                                         # [B,N,N]
    mask8 = np.ascontiguousarray(
        mask8.reshape(B, NT, 128, N))                                  # [B,NT,128,N]
    deg = adj.sum(axis=1)                                              # [B,N]
    corr = (N - deg).astype(np.float32)
    corr = np.ascontiguousarray(
        corr.reshape(B, NT, 128).transpose(0, 2, 1))                   # [B,128,NT]
    wwt16 = np.ascontiguousarray(W_w.T).astype(np.float16)
    amat16 = A.astype(np.float16)
    wb = W_b.reshape(128, 1)
    gwx16 = gate_w[:, :D].astype(np.float16)
    gwaz16 = gate_w[:, D:].astype(np.float16)
    gbvec = np.full((128, 1), gate_b[0], dtype=np.float32)

    in_maps = []
    for c in range(N_CORES):
        gs = slice(c * GPC, (c + 1) * GPC)
        in_maps.append({
            "xt16": xt16[gs], "x16": x16[gs], "mask8": mask8[gs],
            "corr": corr[gs], "wwt16": wwt16, "amat16": amat16, "wb": wb,
            "gwx16": gwx16, "gwaz16": gwaz16, "gbvec": gbvec,
        })

    global _last_in_maps
    _last_in_maps = in_maps
    res = run_bass_kernel_spmd(nc, in_maps, core_ids=list(range(N_CORES)))

    out = np.empty((B, N, D), dtype=np.float32)
    for c in range(N_CORES):
        zo = res.results[c]["zout16"]             # [GPC,2,128,NT//2,128] f16
        zo = np.asarray(zo).astype(np.float32).reshape(GPC, 2, 128, NT // 2, 128)
        zo = zo.transpose(0, 1, 3, 2, 4).reshape(GPC, N, D)
        out[c * GPC:(c + 1) * GPC] = zo
    return out


# revision 27
# speedup vs baseline: 1.0123x; 1.0123x over previous
"""Trainium2 Bass kernel for the GAT-gate GNN forward pass.

Data-parallel over the batch axis: 16 graphs across 8 NeuronCores (2 each).

Per-graph math (N=1024, D=128, NHOP=4), matching reference.py:
    h   = x @ W_w.T + W_b
    hA  = h @ A
    e   = hA @ h.T;  e_sym = e + e.T            (symmetric)
    l   = where(adj>0, e_sym, 0)
    att = softmax(l, axis=0-of-rows) * adj       (softmax over the row index)
    z = h; repeat 4x:
        az = relu(att @ z)
        c  = sigmoid(x @ gw_x + az @ gw_az + gb)
        z  = c*x + (1-c)*az

Device-side structure (v5):
  * ONE PSUM pool: 4 slots of [128,1024] (2 banks each = all 8 banks).
    Phase 1 (h/hA), phase 2 (e_sym), phase 3 (h^T transposes) and phase 4
    (az) all ride the same slot rotation, giving a 4-deep phase-2
    pipeline with no separate-pool fragmentation.
  * e_sym per row-tile r: the additive mask (0 / -1024) is sent as
    fp8e5m2 (exact) and injected into PSUM by two fp8 DoubleRow matmuls
    (106ns each) against constant [I;0] / [0;I] selector weights; two
    fp16 matmuls per 512-chunk accumulate hA_r.h + h_r.hA on top.
    exp flushes masked logits to exactly 0 in fp16; the reference
    denominator's cnt0*exp(0-m) terms are restored from a host-sent
    non-edge count (corr).
  * one merged DVE rowmax reduce [128,1024] (negate=True -> exp bias) and
    one merged ACT exp -> softmax numerators P16 (fp16, <= 1).  Row sums
    come from the exp's accum_out on even rows and a Pool-engine
    sum (scalar_tensor_tensor accum) over P16 on odd rows, balancing ACT
    vs the otherwise idle Pool.  Softmax denominators are folded into the
    per-hop z scaling, so there is no normalize pass over P16.
  * z0 = recip * h^T via 8 PE transposes into a pool slot, evicted with
    per-node scaled copies split between ACT and DVE.
  * per hop/graph/half unit: 32 matmuls -> az PSUM; ACT relu evict (a_h);
    gate dot-products on the Pool engine (scalar_tensor_tensor accum);
    sigmoid on ACT (bias=gb); za = a_h*(recip-cs) split ACT/DVE;
    z' = cs*x + za on DVE.  Units are emitted in ready-time order (each
    unit's za/zw block deferred past the next unit's relu) and the two
    graphs are phase-interleaved so every engine's in-order queue matches
    dependency order and PE never waits.
  * all inputs stream over HWDGE DMAs (SP + one early ACT-issued xt0);
    zout is written fp16 per half and upcast on the host, with stores
    alternating between the SP and Pool DMA queues to parallelize the
    drain.
"""

import sys
import numpy as np

sys.path.insert(0, "/opt/trn_rl_repo")

B, N, D, NHOP = 16, 1024, 128, 4
N_CORES = 8
GPC = B // N_CORES  # graphs per core
NT = N // 128       # 8 node tiles
MASK_NOEDGE = -1024.0

_prog_cache = {}
_last_in_maps = None


def _split_sync_waits(nc, max_waits=1):
    """This walrus build rejects instructions carrying more than a couple of
    sync waits; move excess waits onto same-engine NOPs inserted before the
    offending instruction (per-engine program order is preserved)."""
    import concourse.mybir as mybir

    for fn in nc.m.functions:
        for bb in fn.blocks:
            insts = list(bb.instructions)
            out = []
            changed = False
            for inst in insts:
                si = inst.sync_info
                if si is not None and len(si.on_wait) > max_waits:
                    waits = list(si.on_wait)
                    for w in waits[:-max_waits]:
                        n = mybir.InstNoOp(
                            name=nc.get_next_instruction_name(), ins=[], outs=[]
                        )
                        n.engine = inst.engine
                        n.sync_info = mybir.SyncInfo(on_wait=[w], on_update=[])
                        nc.register_instruction(n)
                        out.append(n)
                    inst.sync_info = mybir.SyncInfo(
                        on_wait=waits[-max_waits:], on_update=list(si.on_update)
                    )
                    changed = True
                out.append(inst)
            if changed:
                bb.instructions = out


def _build_program():
    import concourse.bass as bass
    import concourse.tile as tile
    from concourse import mybir
    from concourse.masks import make_identity
    from contextlib import ExitStack

    f32 = mybir.dt.float32
    f16 = mybir.dt.float16
    f8 = mybir.dt.float8e5
    AF = mybir.ActivationFunctionType
    ALU = mybir.AluOpType
    DR = mybir.MatmulPerfMode.DoubleRow

    nc = bass.Bass("TRN2", target_bir_lowering=False, debug=False)

    # ---- DRAM I/O (per core: GPC graphs) ----
    xt_d = nc.dram_tensor("xt16", [GPC, 128, N], f16, kind="ExternalInput")
    x16_d = nc.dram_tensor("x16", [GPC, 128, NT, 128], f16, kind="ExternalInput")
    mask_d = nc.dram_tensor("mask8", [GPC, NT, 128, N], f8, kind="ExternalInput")
    corr_d = nc.dram_tensor("corr", [GPC, 128, NT], f32, kind="ExternalInput")
    wwt_d = nc.dram_tensor("wwt16", [128, 128], f16, kind="ExternalInput")
    amat_d = nc.dram_tensor("amat16", [128, 128], f16, kind="ExternalInput")
    wb_d = nc.dram_tensor("wb", [128, 1], f32, kind="ExternalInput")
    gwx_d = nc.dram_tensor("gwx16", [1, 128], f16, kind="ExternalInput")
    gwaz_d = nc.dram_tensor("gwaz16", [1, 128], f16, kind="ExternalInput")
    gb_d = nc.dram_tensor("gbvec", [128, 1], f32, kind="ExternalInput")
    zout_d = nc.dram_tensor("zout16", [GPC, 2, 128, NT // 2, 128], f16,
                            kind="ExternalOutput")

    with tile.TileContext(nc) as tc:
        with ExitStack() as ctx:
            singles = ctx.enter_context(tc.tile_pool(name="singles", bufs=1))
            gpool = ctx.enter_context(tc.tile_pool(name="gpool", bufs=1))
            masks = ctx.enter_context(tc.tile_pool(name="masks", bufs=2 * NT))
            zpool = ctx.enter_context(tc.tile_pool(name="zpool", bufs=2))
            scratch = ctx.enter_context(tc.tile_pool(name="scratch", bufs=3))
            zasc = ctx.enter_context(tc.tile_pool(name="zasc", bufs=4))
            tiny = ctx.enter_context(tc.tile_pool(name="tiny", bufs=3))
            psbig = ctx.enter_context(tc.tile_pool(name="psbig", bufs=4, space="PSUM"))

            # ---- input loads: xt0 (split in half for first-matmul latency)
            # on the ACT HWDGE queue ahead of the SP param flood ----
            xt0 = gpool.tile([128, N], f16, tag="xt0", name="xt0")
            nc.scalar.dma_start(xt0[:], xt_d.ap()[0])
            wwt = singles.tile([128, 128], f16)
            nc.sync.dma_start(wwt[:], wwt_d.ap())
            wb = singles.tile([128, 1], f32)
            nc.sync.dma_start(wb[:], wb_d.ap())
            xt1 = gpool.tile([128, N], f16, tag="xt1", name="xt1")
            nc.sync.dma_start(xt1[:], xt_d.ap()[1])
            amat = singles.tile([128, 128], f16)
            nc.sync.dma_start(amat[:], amat_d.ap())
            xt_l = [xt0, xt1]
            mask_t = [[None] * NT for _ in range(GPC)]
            for r in range(2):
                mt = masks.tile([128, N], f8, tag="mask", name="mt")
                nc.sync.dma_start(mt[:], mask_d.ap()[0, r])
                mask_t[0][r] = mt
            gb = singles.tile([128, 1], f32)
            nc.sync.dma_start(gb[:], gb_d.ap())
            # gate weight rows broadcast to all 128 partitions via stride-0 DMA
            gwx = singles.tile([128, 128], f16)
            gwx_src = gwx_d.ap()
            nc.sync.dma_start(
                gwx[:],
                bass.AP(tensor=gwx_src.tensor, offset=gwx_src.offset,
                        ap=[[0, 128]] + list(gwx_src.ap[1:])),
            )
            gwaz = singles.tile([128, 128], f16)
            gwaz_src = gwaz_d.ap()
            nc.sync.dma_start(
                gwaz[:],
                bass.AP(tensor=gwaz_src.tensor, offset=gwaz_src.offset,
                        ap=[[0, 128]] + list(gwaz_src.ap[1:])),
            )
            x16_l, corr_l = [], []
            for g in range(GPC):
                x16 = gpool.tile([128, NT, 128], f16, tag=f"x16{g}",
                                 name=f"x16{g}")
                nc.sync.dma_start(x16[:], x16_d.ap()[g])
                corr = gpool.tile([128, NT], f32, tag=f"corr{g}",
                                  name=f"corr{g}")
                nc.sync.dma_start(corr[:], corr_d.ap()[g])
                x16_l.append(x16)
                corr_l.append(corr)
            for g in range(GPC):
                for r in range(2 if g == 0 else 0, NT):
                    mt = masks.tile([128, N], f8, tag="mask", name="mt")
                    nc.sync.dma_start(mt[:], mask_d.ap()[g, r])
                    mask_t[g][r] = mt

            ident = singles.tile([128, 128], f16)
            make_identity(nc, ident)
            # fp8 DoubleRow selector weights: sel0 = [I; 0], sel1 = [0; I]
            sel0 = singles.tile([128, 2, 128], f8)
            nc.gpsimd.memset(sel0[:], 0)
            nc.gpsimd.tensor_copy(sel0[:, 0, :], ident[:])
            sel1 = singles.tile([128, 2, 128], f8)
            nc.gpsimd.memset(sel1[:], 0)
            nc.gpsimd.tensor_copy(sel1[:, 1, :], ident[:])

            # ---- phase 1: g0's h/hA chain first (it gates phase 2); g1's
            # matmuls fill PE bubbles and its ACT evicts drop into the
            # phase-2 exp pipeline's early gaps ----
            h16_l, ha16_l, gx_l = [], [], []
            for g in range(GPC):
                h16_l.append(gpool.tile([128, N], f16, tag=f"h16{g}",
                                        name=f"h16{g}"))
                ha16_l.append(gpool.tile([128, N], f16, tag=f"ha16{g}",
                                         name=f"ha16{g}"))

            def emit_h(g, evict=True):
                ph = psbig.tile([128, 1024], f32, tag="es", name="ph")
                for cc in range(2):
                    sl = slice(cc * 512, (cc + 1) * 512)
                    nc.tensor.matmul(ph[:, sl], wwt[:], xt_l[g][:, sl],
                                     start=True, stop=True)
                    if evict:
                        nc.scalar.activation(h16_l[g][:, sl], ph[:, sl],
                                             AF.Identity, bias=wb[:],
                                             scale=1.0)
                return ph

            def emit_h_evict(g, ph):
                for cc in range(2):
                    sl = slice(cc * 512, (cc + 1) * 512)
                    nc.scalar.activation(h16_l[g][:, sl], ph[:, sl],
                                         AF.Identity, bias=wb[:], scale=1.0)

            def emit_ha(g):
                pa = psbig.tile([128, 1024], f32, tag="es", name="pa")
                for cc in range(2):
                    sl = slice(cc * 512, (cc + 1) * 512)
                    nc.tensor.matmul(pa[:, sl], amat[:], h16_l[g][:, sl],
                                     start=True, stop=True)
                    nc.scalar.copy(ha16_l[g][:, sl], pa[:, sl])

            emit_h(0)
            emit_ha(0)
            ph1 = emit_h(1, evict=False)

            def emit_gx(g):
                gx_all = tiny.tile([128, NT], f32, tag=f"gx{g}", bufs=1,
                                   name=f"gx{g}")
                for t in range(NT):
                    sc = zasc.tile([128, 128], f16, tag="gsc", name="sc",
                                   bufs=2)
                    nc.vector.scalar_tensor_tensor(
                        out=sc[:], in0=x16_l[g][:, t, :], scalar=1.0, in1=gwx[:],
                        op0=ALU.mult, op1=ALU.mult, accum_out=gx_all[:, t:t + 1])
                gx_l.append(gx_all)

            # ---- phase 2/3 state ----
            p16_l, recip_l, nmax_l, ssum_l, z_l = [], [], [], [], []
            for g in range(GPC):
                p16 = gpool.tile([128, NT, N], f16, tag=f"p16{g}",
                                 name=f"p16{g}")
                p16_l.append(p16)
                recip_l.append(tiny.tile([128, NT], f32, tag=f"recip{g}",
                                         bufs=1, name=f"recip{g}"))
                nmax_l.append(tiny.tile([128, NT], f32, tag=f"nmax{g}",
                                        bufs=1, name=f"nmax{g}"))
                ssum_l.append(tiny.tile([128, NT], f32, tag=f"ssum{g}",
                                        bufs=1, name=f"ssum{g}"))
                z_l.append(None)

            def phase2_inject(g, r):
                # fp8 DoubleRow: es[:, :512] = I.T@mask[:,0,:] + 0,
                #                es[:, 512:] = 0 + I.T@mask[:,1,:]
                mt = mask_t[g][r]
                m2 = mt[:].rearrange("p (two n) -> p two n", two=2)
                es = psbig.tile([128, 1024], f32, tag="es", name="es")
                nc.tensor.matmul(es[:, 0:512], sel0[:], m2,
                                 start=True, stop=False, perf_mode=DR)
                nc.tensor.matmul(es[:, 512:1024], sel1[:], m2,
                                 start=True, stop=False, perf_mode=DR)
                return es

            def phase2_row(g, r, es=None):
                h16, ha16 = h16_l[g], ha16_l[g]
                if es is None:
                    es = phase2_inject(g, r)
                rc = slice(r * 128, (r + 1) * 128)
                for cc in range(2):
                    sl = slice(cc * 512, (cc + 1) * 512)
                    nc.tensor.matmul(es[:, sl], ha16[:, rc], h16[:, sl],
                                     start=False, stop=False)
                    nc.tensor.matmul(es[:, sl], h16[:, rc], ha16[:, sl],
                                     start=False, stop=True)
                nmax = nmax_l[g][:, r:r + 1]
                nc.vector.tensor_reduce(nmax, es[:], mybir.AxisListType.X,
                                        ALU.max, negate=True)
                nc.scalar.activation(p16_l[g][:, r, :], es[:], AF.Exp,
                                     bias=nmax, scale=1.0,
                                     accum_out=ssum_l[g][:, r:r + 1])

            def phase3_den(g):
                # den = ssum + corr * exp(-rowmax)
                expm = tiny.tile([128, NT], f32, tag="expm", name="expm")
                nc.scalar.activation(expm[:], nmax_l[g][:], AF.Exp)
                t0 = tiny.tile([128, NT], f32, tag="t0", name="t0")
                nc.gpsimd.tensor_mul(t0[:], expm[:], corr_l[g][:])
                den = tiny.tile([128, NT], f32, tag="den", name="den")
                nc.gpsimd.tensor_add(den[:], t0[:], ssum_l[g][:])
                nc.vector.reciprocal(recip_l[g][:], den[:])

            def phase3_z0_transposes(g):
                pt = psbig.tile([128, NT, 128], f16, tag="es", name="pt")
                for t in range(NT):
                    nc.tensor.transpose(pt[:, t, :],
                                        h16_l[g][:, t * 128:(t + 1) * 128],
                                        ident[:])
                z_cur = zpool.tile([128, NT, 128], f16, tag=f"z{g}",
                                   name=f"z0_{g}")
                z_l[g] = z_cur
                return pt

            def phase3_z0_evict(g, pt, ts):
                z_cur = z_l[g]
                for t in ts:
                    if t % 2 == 0:
                        nc.scalar.mul(z_cur[:, t, :], pt[:, t, :],
                                      recip_l[g][:, t:t + 1])
                    else:
                        nc.vector.tensor_scalar_mul(z_cur[:, t, :], pt[:, t, :],
                                                    recip_l[g][:, t:t + 1])

            # ---- phase 2 emission: row r0 starts as soon as g0's h/hA are
            # out; g1's phase-1 evictions slot into the exp pipeline gaps ----
            es00 = phase2_inject(0, 0)
            es01 = phase2_inject(0, 1)
            phase2_row(0, 0, es00)
            emit_h_evict(1, ph1)
            phase2_row(0, 1, es01)
            emit_ha(1)
            for r in range(2, NT):
                phase2_row(0, r)
            # g1 rows with g0's phase 3 interleaved into the pipeline tail
            pt0 = None
            for r in range(NT):
                phase2_row(1, r)
                if r == 1:
                    phase3_den(0)
                elif r == 2:
                    pt0 = phase3_z0_transposes(0)
                elif r == 3:
                    phase3_z0_evict(0, pt0, range(0, 4))
                elif r == 4:
                    phase3_z0_evict(0, pt0, range(4, NT))
            emit_gx(0)
            emit_gx(1)
            phase3_den(1)

            # ---- phase 4 ----
            # Per unit (g, half): PE az matmuls; ACT relu evict + sigmoid;
            # DVE gate dot-products (stt accum) + blend writes; Pool the
            # za = a_h*(recip-cs) scalings (walrus allows tensor_scalar with
            # a per-partition pointer on GPSIMD, but not scalar_tensor_tensor
            # or PSUM access).  Each engine's in-order queue then matches
            # dependency order naturally.
            def emit_unit(g, half, z_next, last, interleave=None):
                p16, gx_all, x16 = p16_l[g], gx_l[g], x16_l[g]
                z_cur = z_l[g]
                az_h = psbig.tile([128, 512], f32, tag="es", name="az")
                for i2 in range(4):
                    i = half * 4 + i2
                    sl = slice(i2 * 128, (i2 + 1) * 128)
                    for j in range(NT):
                        nc.tensor.matmul(az_h[:, sl],
                                         p16[:, j, i * 128:(i + 1) * 128],
                                         z_cur[:, j, :],
                                         start=(j == 0),
                                         stop=(j == NT - 1))
                if interleave is not None:
                    interleave()
                final = last and g == 1 and half == 1
                a_h = scratch.tile([128, 512], f16, tag="a_h", name="a_h")
                nc.scalar.activation(a_h[:], az_h[:], AF.Relu)
                gaz_h = tiny.tile([128, 4], f32, tag="gazh", name="gazh")
                for i2 in range(4):
                    sc = zasc.tile([128, 128], f16, tag="gsc", name="sc",
                                   bufs=2)
                    nc.vector.scalar_tensor_tensor(
                        out=sc[:], in0=a_h[:, i2 * 128:(i2 + 1) * 128],
                        scalar=1.0, in1=gwaz[:],
                        op0=ALU.mult, op1=ALU.mult,
                        accum_out=gaz_h[:, i2:i2 + 1])
                hs = slice(half * 4, half * 4 + 4)
                sig_in = tiny.tile([128, 4], f32, tag="sigin", name="sigin")
                nc.vector.tensor_add(sig_in[:], gaz_h[:], gx_all[:, hs])
                c_h = tiny.tile([128, 4], f32, tag="c_h", name="c_h")
                nc.scalar.activation(c_h[:], sig_in[:], AF.Sigmoid, bias=gb[:])
                if not last:
                    cs = tiny.tile([128, 4], f32, tag="cs", name="cs")
                    nc.vector.tensor_mul(cs[:], c_h[:], recip_l[g][:, hs])
                    cm1s = tiny.tile([128, 4], f32, tag="cm1s", name="cm1s")
                    nc.vector.tensor_sub(cm1s[:], recip_l[g][:, hs], cs[:])
                else:
                    cs = c_h
                    cm1s = tiny.tile([128, 4], f32, tag="cm1s", name="cm1s")
                    nc.vector.tensor_scalar(
                        out=cm1s[:], in0=c_h[:], scalar1=-1.0,
                        scalar2=1.0, op0=ALU.mult, op1=ALU.add)
                for i2 in range(4):
                    i = half * 4 + i2
                    asl = a_h[:, i2 * 128:(i2 + 1) * 128]
                    # blend engine assignment per chunk (balance ACT/DVE/Pool;
                    # plain tensor_scalar gets DVE 4x mode, stt does not):
                    #   i2=0: za/cx/add all Pool   i2=1: za ACT + stt DVE
                    #   i2=2: za DVE + stt DVE     i2=3: za DVE + cx/add Pool
                    if final:
                        za = zasc.tile([128, 128], f16, tag="zad", name="za")
                        nc.vector.tensor_scalar_mul(za[:], asl,
                                                    cm1s[:, i2:i2 + 1])
                        nc.vector.scalar_tensor_tensor(
                            out=z_next[:, i, :], in0=x16[:, i, :],
                            scalar=cs[:, i2:i2 + 1],
                            in1=za[:], op0=ALU.mult, op1=ALU.add)
                        continue
                    if i2 == 0:
                        za = zasc.tile([128, 128], f16, tag="zap", name="za")
                        nc.gpsimd.tensor_scalar_mul(za[:], asl,
                                                    cm1s[:, i2:i2 + 1])
                        cx = zasc.tile([128, 128], f16, tag="cx", name="cx",
                                       bufs=2)
                        nc.gpsimd.tensor_scalar_mul(cx[:], x16[:, i, :],
                                                    cs[:, i2:i2 + 1])
                        nc.gpsimd.tensor_add(z_next[:, i, :], cx[:], za[:])
                    elif i2 == 1:
                        za = zasc.tile([128, 128], f16, tag="zaa", name="za")
                        nc.scalar.mul(za[:], asl, cm1s[:, i2:i2 + 1])
                        nc.vector.scalar_tensor_tensor(
                            out=z_next[:, i, :], in0=x16[:, i, :],
                            scalar=cs[:, i2:i2 + 1],
                            in1=za[:], op0=ALU.mult, op1=ALU.add)
                    elif i2 == 2:
                        za = zasc.tile([128, 128], f16, tag="zad", name="za")
                        nc.vector.tensor_scalar_mul(za[:], asl,
                                                    cm1s[:, i2:i2 + 1])
                        nc.vector.scalar_tensor_tensor(
                            out=z_next[:, i, :], in0=x16[:, i, :],
                            scalar=cs[:, i2:i2 + 1],
                            in1=za[:], op0=ALU.mult, op1=ALU.add)
                    else:
                        za = zasc.tile([128, 128], f16, tag="zad", name="za")
                        nc.vector.tensor_scalar_mul(za[:], asl,
                                                    cm1s[:, i2:i2 + 1])
                        cx = zasc.tile([128, 128], f16, tag="cx", name="cx",
                                       bufs=2)
                        nc.gpsimd.tensor_scalar_mul(cx[:], x16[:, i, :],
                                                    cs[:, i2:i2 + 1])
                        nc.gpsimd.tensor_add(z_next[:, i, :], cx[:], za[:])
                if last:
                    nc.sync.dma_start(zout_d.ap()[g, half],
                                      z_next[:, half * 4:(half + 1) * 4, :])

            pt1 = None
            for t_hop in range(NHOP):
                last = t_hop == NHOP - 1
                z_next_l = []
                for g in range(GPC):
                    if last:
                        z_next_l.append(gpool.tile([128, NT, 128], f16,
                                                   tag=f"zo{g}",
                                                   name=f"zo{g}"))
                    else:
                        z_next_l.append(zpool.tile([128, NT, 128], f16,
                                                   tag=f"z{g}",
                                                   name=f"zn{g}"))
                for g in range(GPC):
                    for half in range(2):
                        # hop 0 rides the phase-2/3 tail: g1's z0 work is
                        # interleaved between g0's first az units
                        interleave = None
                        if t_hop == 0 and g == 0 and half == 0:
                            def interleave():
                                nonlocal pt1
                                pt1 = phase3_z0_transposes(1)
                        elif t_hop == 0 and g == 0 and half == 1:
                            def interleave():
                                phase3_z0_evict(1, pt1, range(NT))
                        emit_unit(g, half, z_next_l[g], last, interleave)
                    if not last:
                        z_l[g] = z_next_l[g]

    _split_sync_waits(nc)
    return nc


def kernel(x, adj, W_w, W_b, A, gate_w, gate_b):
    import ml_dtypes
    from concourse.bass_utils import run_bass_kernel_spmd

    x = np.asarray(x, dtype=np.float32)
    adj = np.asarray(adj, dtype=np.float32)
    W_w = np.asarray(W_w, dtype=np.float32)
    W_b = np.asarray(W_b, dtype=np.float32)
    A = np.asarray(A, dtype=np.float32)
    gate_w = np.asarray(gate_w, dtype=np.float32)
    gate_b = np.asarray(gate_b, dtype=np.float32)

    if "nc" not in _prog_cache:
        _prog_cache["nc"] = _build_program()
    nc = _prog_cache["nc"]

    # ---- host-side prep ----
    xt16 = np.ascontiguousarray(x.transpose(0, 2, 1)).astype(np.float16)
    x16 = np.ascontiguousarray(
        x.reshape(B, NT, 128, D).transpose(0, 2, 1, 3)).astype(np.float16)
    mask8 = np.where(adj > 0.0, 0.0, MASK_NOEDGE).astype(
        ml_dtypes.float8_e5m2)                                         # [B,N,N]
    mask8 = np.ascontiguousarray(
        mask8.reshape(B, NT, 128, N))                                  # [B,NT,128,N]
    deg = adj.sum(axis=1)                                              # [B,N]
    corr = (N - deg).astype(np.float32)
    corr = np.ascontiguousarray(
        corr.reshape(B, NT, 128).transpose(0, 2, 1))                   # [B,128,NT]
    wwt16 = np.ascontiguousarray(W_w.T).astype(np.float16)
    amat16 = A.astype(np.float16)
    wb = W_b.reshape(128, 1)
    gwx16 = gate_w[:, :D].astype(np.float16)
    gwaz16 = gate_w[:, D:].astype(np.float16)
    gbvec = np.full((128, 1), gate_b[0], dtype=np.float32)

    in_maps = []
    for c in range(N_CORES):
        gs = slice(c * GPC, (c + 1) * GPC)
        in_maps.append({
            "xt16": xt16[gs], "x16": x16[gs], "mask8": mask8[gs],
            "corr": corr[gs], "wwt16": wwt16, "amat16": amat16, "wb": wb,
            "gwx16": gwx16, "gwaz16": gwaz16, "gbvec": gbvec,
        })

    global _last_in_maps
    _last_in_maps = in_maps
    res = run_bass_kernel_spmd(nc, in_maps, core_ids=list(range(N_CORES)))

    out = np.empty((B, N, D), dtype=np.float32)
    for c in range(N_CORES):
        zo = res.results[c]["zout16"]             # [GPC,2,128,NT//2,128] f16
        zo = np.asarray(zo).astype(np.float32).reshape(GPC, 2, 128, NT // 2, 128)
        zo = zo.transpose(0, 1, 3, 2, 4).reshape(GPC, N, D)
        out[c * GPC:(c + 1) * GPC] = zo
    return out
